# revision 23
# baseline (speedup 1.0000x reference)
"""Trainium2 Bass kernel for nn_BondLenConstrain (v2, fp16 pipeline).

Contract: kernel(**inputs) takes the FULL (unsharded) inputs of
reference.setup_inputs() and returns the full [64, 4, 2048, 2] float32
resiEnergy tensor.  Data-parallel over the batch axis across 8 NeuronCores
(8 batches per core).

Host (numpy, indexing only): scatter atoms into dense residue grids,
build the `todo` mask, gather the tiny per-residue-type tables into
per-pair fp16 coefficient planes (masked pairs get all-zero coefficients
-> device returns exactly 0), and lay out coords (f32, prescaled by 1/16)
in a plane-contiguous blocked layout with a one-slot halo.

Device math per residue pair (r-1, r), fp16 unless noted:
    v2 = CA_r - N_r, v1 = C_{r-1} - N_r, v3 = CA_{r-1} - C_{r-1}
    (subtractions read f32 coords, write fp16 - avoids cancellation loss)
    d11,d22,d33,c1,c2 via one self-mul + one cross-mul + two segmented adds
    ln pass (f32 out) over [d11 | s1^2 | s2^2 | |c1| | |c2|]
    f1 = exp(0.5 ln d11)
    L = ln|c| - 0.5 ln(s^2)  (= ln t, t = |c|/s)   [f32]
    phi = pi/4 + arctan(tanh(L/2))    <- Gudermannian identity replaces the
        sign/exp range-reduction dance: arctan(t) = pi/4 + arctan(tanh(ln(t)/2))
    U = [f1*QB - mu0*Q0 | phi*(+-Q) - sign(c)*(pi/2-mu)*Q]   (x-> -sx, squared)
    E = sum_d min(U^2, CAP)
ACT function tables: free-running per-chunk order (ln -> exp -> tanh ->
arctan); square/sign/abs ride every set.  A dummy Ln hides the first
table load inside the DMA fill.  Forcing globally grouped table order
measured slower (serializes the per-chunk tails past the last front).
Sign is computed on DVE (two 4x tensor_scalar ops) so no const AP or
preamble barrier is needed.  The degenerate-geometry clamps are off by
default (BLC_CLAMPS=0): s^2<=0 / c=0 produce NaN/-inf which flow through
tanh/arctan to the capped branch (DVE min returns the non-NaN operand),
matching the reference's capped score exactly.
"""

import os
import numpy as np

PAD = -999.0
PAD_I = -999
NB, MC, MR = 64, 4, 2048
NALT = 2
NCORES = 8
BPC = NB // NCORES            # batches per core
CH = int(os.environ.get("BLC_CHUNKS", "2"))  # pipeline chunks per core
KC = 4 * CH                   # blocks per (batch, chain) across full chain
R = MR // KC                  # residues (pairs) per partition
S = R + 1                     # coord slots per plane (halo)
EPS = 1e-12
CL = 1.0 / (EPS * np.sqrt(np.pi))
SC = 1.0 / 16.0               # coord prescale (fp16 range safety)
CLAMPS = bool(int(os.environ.get("BLC_CLAMPS", "0")))

_PROGRAM_CACHE = {}
LAST_RESULT = None            # BassKernelResults of the last run (for test.py)
TRACE = bool(int(os.environ.get("BLC_TRACE", "0")))


def _build_program():
    import concourse.bass as bass
    import concourse.tile as tile
    from concourse import bacc, mybir
    from concourse.bass import _add_dep_helper
    f16 = mybir.dt.float16
    f32 = mybir.dt.float32
    Alu = mybir.AluOpType
    Act = mybir.ActivationFunctionType

    nc = bacc.Bacc("TRN2", target_bir_lowering=False, debug=False)

    X_t = nc.declare_dram_parameter("cx", [BPC, MC, KC, 9, S], f32,
                                    isOutput=False)
    P_t = nc.declare_dram_parameter("pr", [BPC, MC, KC, 9, R], f16,
                                    isOutput=False)
    O_t = nc.declare_dram_parameter("out", [BPC, MC, MR], f16, isOutput=True)


    bc = BPC // CH            # batches per chunk
    bufs = min(CH, 2)
    PI4 = float(np.pi / 4)

    with tile.TileContext(nc) as tc:
        with (
            tc.tile_pool(name="px", bufs=bufs) as px,
            tc.tile_pool(name="ps", bufs=bufs) as ps,
        ):
            # ---- DMA loads for all chunks up front (one sync ring) -------
            # coords first (they gate the fronts); coefficient planes are
            # only needed by the tails, so they stream last
            loads = []
            for c in range(CH):
                b0 = c * bc
                X = px.tile([128, 9 * S], f32, tag="x")
                P = px.tile([128, 9 * R], f16, tag="p")
                # split the coord load: N+CA planes first so the v2
                # subtraction can start while C planes still stream
                nc.sync.dma_start(X[:, 0:6 * S], X_t[b0:b0 + bc, :, :, 0:6])
                nc.sync.dma_start(X[:, 6 * S:9 * S],
                                  X_t[b0:b0 + bc, :, :, 6:9])
                loads.append((X, P))
            for c in range(CH):
                b0 = c * bc
                X, P = loads[c]
                nc.sync.dma_start(P[:], P_t[b0:b0 + bc])

            # dummy Ln on a tiny scratch tile: preloads the {ln} table
            # under the DMA fill (input value is irrelevant)
            dscr = ps.tile([128, 1], f32, tag="dummy")
            nc.vector.memset(dscr[:], 1.0)
            dummy_ln = nc.scalar.activation(dscr[:], dscr[:], Act.Ln)
            tblchain = [dummy_ln]

            def tbl_order(bi):
                _add_dep_helper(bi.ins, tblchain[-1].ins, sync=False,
                                reason="act-table-order")
                tblchain.append(bi)
                return bi

            # ---- phase 1 (front): geometry up to the ln, per chunk -------
            st = []
            for c in range(CH):
                X, P = loads[c]
                Xv = X[:].rearrange("p (n l) -> p n l", n=9)
                # c-major: V[p, c, v, l], v in {v2, v1, v3} -> the three
                # per-coordinate SQCP blocks become contiguous 5R adds
                V = px.tile([128, 9 * R], f16, tag="v")
                Vv = V[:].rearrange("p (cc v l) -> p cc v l", cc=3, v=3)
                nc.vector.tensor_sub(Vv[:, :, 0], Xv[:, 3:6, 1:S],
                                     Xv[:, 0:3, 1:S])
                nc.vector.tensor_sub(Vv[:, :, 1], Xv[:, 6:9, 0:R],
                                     Xv[:, 0:3, 1:S])
                nc.vector.tensor_sub(Vv[:, :, 2], Xv[:, 3:6, 0:R],
                                     Xv[:, 6:9, 0:R])

                SQCP = px.tile([128, 15 * R], f16, tag="sqcp")
                Qv = SQCP[:].rearrange("p (cc g l) -> p cc g l", cc=3, g=5)
                if c == 0:
                    # chunk 0 has nothing to overlap its front: the ACT
                    # square would stall DVE; keep it on DVE there
                    nc.vector.tensor_mul(Qv[:, :, 0:3], Vv[:], Vv[:])
                else:
                    nc.scalar.square(Qv[:, :, 0:3], Vv[:])
                nc.vector.tensor_mul(Qv[:, :, 3:5], Vv[:, :, 1:3],
                                     Vv[:, :, 0:2])
                DD = ps.tile([128, 5 * R], f16, tag="dd")  # [d22|d11|d33|c1|c2]
                nc.vector.tensor_add(DD[:], SQCP[:, 0:5 * R],
                                     SQCP[:, 5 * R:10 * R])
                nc.vector.tensor_add(DD[:], DD[:], SQCP[:, 10 * R:15 * R])

                LNIN = ps.tile([128, 5 * R], f16, tag="lnin")
                MT = ps.tile([128, 2 * R], f16, tag="mt")   # [m1 | m2]
                nc.vector.tensor_mul(MT[:, 0:R], DD[:, 0:R], DD[:, R:2 * R])
                nc.vector.tensor_mul(MT[:, R:2 * R], DD[:, 2 * R:3 * R],
                                     DD[:, R:2 * R])
                PSQ = ps.tile([128, 2 * R], f16, tag="psq")  # [c1^2 | c2^2]
                nc.vector.tensor_mul(PSQ[:], DD[:, 3 * R:5 * R],
                                     DD[:, 3 * R:5 * R])
                SG = ps.tile([128, 2 * R], f16, tag="sg")
                nc.vector.tensor_scalar(SG[:], DD[:, 3 * R:5 * R], 0.0, None,
                                        op0=Alu.is_ge)
                nc.vector.tensor_scalar(SG[:], SG[:], 2.0, -1.0,
                                        op0=Alu.mult, op1=Alu.add)
                nc.scalar.activation(LNIN[:, R:3 * R], DD[:, 3 * R:5 * R],
                                     Act.Abs)
                nc.vector.tensor_scalar_max(LNIN[:, 0:R], DD[:, R:2 * R],
                                            6e-8)
                LNO = ps.tile([128, 5 * R], f32, tag="lno")
                nc.vector.tensor_sub(LNIN[:, 3 * R:5 * R], MT[:], PSQ[:])
                if CLAMPS:
                    nc.vector.tensor_scalar_max(LNIN[:, R:3 * R],
                                                LNIN[:, R:3 * R], 1e-6)
                    nc.vector.tensor_scalar_max(LNIN[:, 3 * R:5 * R],
                                                LNIN[:, 3 * R:5 * R], 6e-8)
                tbl_order(nc.scalar.activation(LNO[:], LNIN[:], Act.Ln))
                st.append((P, SG, LNO))

            # ---- phase 2 (tail): angle + scoring, per chunk --------------
            # ACT order: Exp0, Tanh0, Exp1, Tanh1 (one exp_and_others load,
            # tanh rides it), then Arctan0..1 (one sigmoid_and_others load,
            # the U^2 squares ride that)
            tails = []
            for c in range(CH):
                P, SG, LNO = st[c]
                L = ps.tile([128, 2 * R], f32, tag="l")
                nc.vector.scalar_tensor_tensor(
                    L[:], LNO[:, 3 * R:5 * R], -0.5, LNO[:, R:3 * R],
                    op0=Alu.mult, op1=Alu.add)
                AV = ps.tile([128, 2 * R], f16, tag="av")
                nc.gpsimd.tensor_mul(AV[:], SG[:], P[:, 4 * R:6 * R])
                T3 = ps.tile([128, 3 * R], f16, tag="t3")   # [f1 | phi1 | phi2]
                tbl_order(nc.scalar.activation(T3[:, 0:R], LNO[:, 0:R],
                                                Act.Exp, scale=0.5))
                TH = ps.tile([128, 2 * R], f16, tag="th")
                tbl_order(nc.scalar.activation(TH[:], L[:], Act.Tanh,
                                               scale=0.5))
                tails.append((L, AV, TH, T3))

            # bond leg (needs only exp's f1): fills the DVE idle window
            # while tanh/arctan + their table loads run on ACT
            zcs = []
            for c in range(CH):
                P, SG, LNO = st[c]
                L, AV, TH, T3 = tails[c]
                ZC = ps.tile([128, 3 * R], f16, tag="zc")
                WB = ps.tile([128, R], f16, tag="wb")
                nc.vector.tensor_mul(WB[:], T3[:, 0:R], P[:, 0:R])
                nc.vector.tensor_sub(WB[:], WB[:], P[:, 3 * R:4 * R])
                nc.vector.tensor_mul(WB[:], WB[:], WB[:])
                nc.vector.tensor_tensor(ZC[:, 0:R], WB[:], P[:, 6 * R:7 * R],
                                        op=Alu.min)
                zcs.append(ZC)

            for c in range(CH):
                b0 = c * bc
                P, SG, LNO = st[c]
                L, AV, TH, T3 = tails[c]
                ZC = zcs[c]
                tbl_order(nc.scalar.activation(T3[:, R:3 * R], TH[:],
                                               Act.Arctan))
                nc.vector.tensor_scalar_add(T3[:, R:3 * R], T3[:, R:3 * R],
                                            PI4)
                W = ps.tile([128, 2 * R], f16, tag="w")
                nc.vector.tensor_mul(W[:], T3[:, R:3 * R], P[:, R:3 * R])
                U = ps.tile([128, 2 * R], f16, tag="u")
                nc.vector.tensor_sub(U[:], W[:], AV[:])
                nc.vector.tensor_mul(U[:], U[:], U[:])
                nc.vector.tensor_tensor(ZC[:, R:3 * R], U[:],
                                        P[:, 7 * R:9 * R], op=Alu.min)
                E = ps.tile([128, R], f16, tag="e")
                nc.vector.tensor_add(E[:], ZC[:, 0:R], ZC[:, R:2 * R])
                nc.vector.tensor_add(E[:], E[:], ZC[:, 2 * R:3 * R])
                nc.sync.dma_start(
                    O_t[b0:b0 + bc].rearrange("b c (k l) -> b c k l", k=KC),
                    E[:])

    return nc


def _strip_auto_act_loads(nc):
    """Drop the table loads Bacc's insert_act_table_loads added: its
    first-match set choice ping-pongs between {ln}/{exp}/{arctan} sets.
    Our two manual loads (ln+exp set, tanh+arctan set) cover every
    activation in program order.  The pass runs after semaphore
    generation, so its loads carry no sync info and are safe to remove."""
    from concourse import mybir
    manual = getattr(nc, "_manual_act_loads", set())
    removed = 0
    for f in nc.m.functions:
        for blk in f.blocks:
            keep = []
            for inst in blk.instructions:
                if (isinstance(inst, mybir.InstLoadActFuncSet)
                        and inst.name not in manual):
                    si = inst.sync_info
                    if si is not None and (len(si.on_wait) or len(si.on_update)):
                        keep.append(inst)  # has sync; leave it alone
                        continue
                    removed += 1
                    continue
                keep.append(inst)
            blk.instructions[:] = keep
    return removed


def _get_program():
    if "nc" not in _PROGRAM_CACHE:
        nc = _build_program()
        nc.finalize()   # Bacc: register allocation / DCE / wait legalization
        if bool(int(os.environ.get("BLC_STRIP_LOADS", "0"))):
            _strip_auto_act_loads(nc)
        _PROGRAM_CACHE["nc"] = nc
    return _PROGRAM_CACHE["nc"]


def _host_prep(atom_description, coords, mean, std, weight):
    ad = np.asarray(atom_description)
    coords = np.asarray(coords, dtype=np.float32)
    b, ch, rs, rn, an = (ad[:, i] for i in range(5))
    valid = (b >= 0) & (b < NB) & (ch >= 0) & (ch < MC) & (rs >= 0) & (rs < MR)

    def scat3(mask):
        A = np.full((NB, MC, MR, 3), PAD, np.float32)
        m = mask & valid
        A[b[m], ch[m], rs[m]] = coords[m]
        return A

    Narr, CAarr, Carr = scat3(an == 0), scat3(an == 1), scat3(an == 2)
    seq = np.full((NB, MC, MR), PAD_I, np.int64)
    m = (an == 1) & valid
    seq[b[m], ch[m], rs[m]] = rn[m]

    todo = ((Narr[:, :, 1:, 0] != PAD) & (Carr[:, :, :-1, 0] != PAD)
            & (CAarr[:, :, 1:, 0] != PAD) & (CAarr[:, :, :-1, 0] != PAD)
            & (seq[:, :, 1:] != PAD_I) & (seq[:, :, :-1] != PAD_I))
    sidx = np.clip(np.where(todo, seq[:, :, 1:], 0), 0, 19)

    w0 = float(np.asarray(weight).reshape(-1)[0])
    s_w = 1.0 - np.tanh(-w0)
    sqw = np.sqrt(s_w)
    mu = np.asarray(mean, np.float64)
    sd = np.asarray(std, np.float64)
    qd = 1.0 / (sd * np.sqrt(2.0))
    Q = qd * sqw
    tab = np.empty((20, 9))
    tab[:, 0] = (1.0 / SC) * Q[:, 0]            # QB
    tab[:, 1] = Q[:, 1]                         # QS1
    tab[:, 2] = -Q[:, 2]                        # QS2
    tab[:, 3] = mu[:, 0] * Q[:, 0]              # MU0*Q0
    tab[:, 4] = (np.pi / 2 - mu[:, 1]) * Q[:, 1]  # MQ1
    tab[:, 5] = (np.pi / 2 - mu[:, 2]) * Q[:, 2]  # MQ2
    tab[:, 6:9] = s_w * np.maximum(np.log(CL * qd), 0.0)  # CAP
    tab = tab.astype(np.float32)

    params = np.zeros((NB, MC, MR, 9), np.float32)
    params[:, :, 1:, :] = tab[sidx] * todo[..., None].astype(np.float32)
    pblk = np.ascontiguousarray(
        params.reshape(NB, MC, KC, R, 9).transpose(0, 1, 2, 4, 3)
    ).astype(np.float16)

    G = np.zeros((NB, MC, MR + 1, 9), np.float32)
    G[:, :, 1:, 0:3] = np.where(Narr == PAD, 0.0, Narr) * SC
    G[:, :, 1:, 3:6] = np.where(CAarr == PAD, 0.0, CAarr) * SC
    G[:, :, 1:, 6:9] = np.where(Carr == PAD, 0.0, Carr) * SC
    # blocked plane-contiguous with halo: GB[b,c,k,p,l] = G[b,c,k*R+l,p]
    GB = np.empty((NB, MC, KC, 9, S), np.float32)
    for k in range(KC):
        GB[:, :, k] = G[:, :, k * R:k * R + S, :].transpose(0, 1, 3, 2)
    return GB, pblk


def _install_ntff_hook():
    """The agent image's antenv lacks axon_hooks; synthesize it so
    trace=True can reach the terminal's NRT profiler (dev-only path)."""
    import sys, types
    if "antenv.axon_hooks" in sys.modules:
        return True
    try:
        import antenv
        mod = types.ModuleType("antenv.axon_hooks")
        mod._hook = None

        def set_axon_ntff_profile_hook(h):
            mod._hook = h

        def get_axon_ntff_profile_hook():
            return mod._hook

        mod.set_axon_ntff_profile_hook = set_axon_ntff_profile_hook
        mod.get_axon_ntff_profile_hook = get_axon_ntff_profile_hook
        sys.modules["antenv.axon_hooks"] = mod
        antenv.axon_hooks = mod
        from trn_agent_boot.trn_boot import _ntff_profile_via_ctypes
        mod._hook = _ntff_profile_via_ctypes("/opt/axon/libaxon_pjrt.so")
        return True
    except Exception as e:  # pragma: no cover - profiling is best-effort
        print(f"ntff hook install failed: {e}")
        return False


def kernel(**inputs):
    global LAST_RESULT
    from concourse.bass_utils import run_bass_kernel_spmd
    if TRACE:
        _install_ntff_hook()

    G, pblk = _host_prep(
        inputs["atom_description"], inputs["coords"],
        inputs["mean"], inputs["std"], inputs["weight"])

    nc = _get_program()
    in_maps = [
        {"cx": np.ascontiguousarray(G[i * BPC:(i + 1) * BPC]),
         "pr": np.ascontiguousarray(pblk[i * BPC:(i + 1) * BPC])}
        for i in range(NCORES)
    ]
    res = run_bass_kernel_spmd(nc, in_maps, list(range(NCORES)), trace=TRACE)
    LAST_RESULT = res
    e = np.concatenate([res.results[i]["out"] for i in range(NCORES)], axis=0)
    e = e.astype(np.float32).reshape(NB, MC, MR)
    out = np.repeat(e[..., None], NALT, axis=-1)
    return np.ascontiguousarray(out.astype(np.float32))


# revision 24
# speedup vs baseline: 1.0956x; 1.0956x over previous
"""Trainium2 Bass kernel for nn_BondLenConstrain (v2, fp16 pipeline).

Contract: kernel(**inputs) takes the FULL (unsharded) inputs of
reference.setup_inputs() and returns the full [64, 4, 2048, 2] float32
resiEnergy tensor.  Data-parallel over the batch axis across 8 NeuronCores
(8 batches per core).

Host (numpy, indexing only): scatter atoms into dense residue grids,
build the `todo` mask, gather the tiny per-residue-type tables into
per-pair fp16 coefficient planes (masked pairs get all-zero coefficients
-> device returns exactly 0), and lay out coords (f32, prescaled by 1/16)
in a plane-contiguous blocked layout with a one-slot halo.

Device math per residue pair (r-1, r), fp16 unless noted:
    v2 = CA_r - N_r, v1 = C_{r-1} - N_r, v3 = CA_{r-1} - C_{r-1}
    (subtractions read f32 coords, write fp16 - avoids cancellation loss)
    d11,d22,d33,c1,c2 via one self-mul + one cross-mul + two segmented adds
    ln pass (f32 out) over [d11 | s1^2 | s2^2 | |c1| | |c2|]
    f1 = exp(0.5 ln d11)
    L = ln|c| - 0.5 ln(s^2)  (= ln t, t = |c|/s)   [f32]
    phi = pi/4 + arctan(tanh(L/2))    <- Gudermannian identity replaces the
        sign/exp range-reduction dance: arctan(t) = pi/4 + arctan(tanh(ln(t)/2))
    U = [f1*QB - mu0*Q0 | phi*(+-Q) - sign(c)*(pi/2-mu)*Q]   (x-> -sx, squared)
    E = sum_d min(U^2, CAP)
ACT function tables: free-running per-chunk order (ln -> exp -> tanh ->
arctan); square/sign/abs ride every set.  A dummy Ln hides the first
table load inside the DMA fill.  Forcing globally grouped table order
measured slower (serializes the per-chunk tails past the last front).
Sign is computed on DVE (two 4x tensor_scalar ops) so no const AP or
preamble barrier is needed.  The degenerate-geometry clamps are off by
default (BLC_CLAMPS=0): s^2<=0 / c=0 produce NaN/-inf which flow through
tanh/arctan to the capped branch (DVE min returns the non-NaN operand),
matching the reference's capped score exactly.
"""

import os
import numpy as np

PAD = -999.0
PAD_I = -999
NB, MC, MR = 64, 4, 2048
NALT = 2
NCORES = 8
BPC = NB // NCORES            # batches per core
CH = int(os.environ.get("BLC_CHUNKS", "2"))  # pipeline chunks per core
KC = 4 * CH                   # blocks per (batch, chain) across full chain
R = MR // KC                  # residues (pairs) per partition
S = R + 1                     # coord slots per plane (halo)
EPS = 1e-12
CL = 1.0 / (EPS * np.sqrt(np.pi))
SC = 1.0 / 16.0               # coord prescale (fp16 range safety)
CLAMPS = bool(int(os.environ.get("BLC_CLAMPS", "0")))

_PROGRAM_CACHE = {}
LAST_RESULT = None            # BassKernelResults of the last run (for test.py)
TRACE = bool(int(os.environ.get("BLC_TRACE", "0")))


def _build_program():
    import concourse.bass as bass
    import concourse.tile as tile
    from concourse import bacc, mybir
    from concourse.bass import _add_dep_helper
    f16 = mybir.dt.float16
    f32 = mybir.dt.float32
    Alu = mybir.AluOpType
    Act = mybir.ActivationFunctionType

    nc = bacc.Bacc("TRN2", target_bir_lowering=False, debug=False)

    X_t = nc.declare_dram_parameter("cx", [BPC, MC, KC, 9, S], f32,
                                    isOutput=False)
    P_t = nc.declare_dram_parameter("pr", [BPC, MC, KC, 9, R], f16,
                                    isOutput=False)
    O_t = nc.declare_dram_parameter("out", [BPC, MC, MR], f16, isOutput=True)


    bc = BPC // CH            # batches per chunk
    bufs = min(CH, 2)
    PI4 = float(np.pi / 4)

    with tile.TileContext(nc) as tc:
        with (
            tc.tile_pool(name="px", bufs=bufs) as px,
            tc.tile_pool(name="ps", bufs=bufs) as ps,
        ):
            # ---- DMA loads for all chunks up front (one sync ring) -------
            # coords first (they gate the fronts); coefficient planes are
            # only needed by the tails, so they stream last
            loads = []
            for c in range(CH):
                b0 = c * bc
                X = px.tile([128, 9 * S], f32, tag="x")
                P = px.tile([128, 9 * R], f16, tag="p")
                # split the coord load: N+CA planes first so the v2
                # subtraction can start while C planes still stream
                nc.sync.dma_start(X[:, 0:6 * S], X_t[b0:b0 + bc, :, :, 0:6])
                nc.sync.dma_start(X[:, 6 * S:9 * S],
                                  X_t[b0:b0 + bc, :, :, 6:9])
                loads.append((X, P))
            for c in range(CH):
                b0 = c * bc
                X, P = loads[c]
                nc.sync.dma_start(P[:], P_t[b0:b0 + bc])

            # dummy Ln on a tiny scratch tile: preloads the {ln} table
            # under the DMA fill (input value is irrelevant)
            dscr = ps.tile([128, 1], f32, tag="dummy")
            nc.vector.memset(dscr[:], 1.0)
            dummy_ln = nc.scalar.activation(dscr[:], dscr[:], Act.Ln)
            tblchain = [dummy_ln]

            def tbl_order(bi):
                _add_dep_helper(bi.ins, tblchain[-1].ins, sync=False,
                                reason="act-table-order")
                tblchain.append(bi)
                return bi

            # ---- phase 1a: difference vectors + products, per chunk ------
            # (squares ride ACT while DVE moves on to the next chunk's subs)
            geo = []
            for c in range(CH):
                X, P = loads[c]
                Xv = X[:].rearrange("p (n l) -> p n l", n=9)
                # c-major: V[p, c, v, l], v in {v2, v1, v3} -> the three
                # per-coordinate SQCP blocks become contiguous 5R adds
                V = px.tile([128, 9 * R], f16, tag="v")
                Vv = V[:].rearrange("p (cc v l) -> p cc v l", cc=3, v=3)
                nc.vector.tensor_sub(Vv[:, :, 0], Xv[:, 3:6, 1:S],
                                     Xv[:, 0:3, 1:S])
                nc.vector.tensor_sub(Vv[:, :, 1], Xv[:, 6:9, 0:R],
                                     Xv[:, 0:3, 1:S])
                nc.vector.tensor_sub(Vv[:, :, 2], Xv[:, 3:6, 0:R],
                                     Xv[:, 6:9, 0:R])
                SQCP = px.tile([128, 15 * R], f16, tag="sqcp")
                Qv = SQCP[:].rearrange("p (cc g l) -> p cc g l", cc=3, g=5)
                nc.vector.tensor_mul(Qv[:, :, 3:5], Vv[:, :, 1:3],
                                     Vv[:, :, 0:2])
                nc.scalar.square(Qv[:, :, 0:3], Vv[:])
                geo.append((V, SQCP))

            # ---- phase 1b: reductions down to the ln, per chunk ----------
            st = []
            for c in range(CH):
                X, P = loads[c]
                V, SQCP = geo[c]
                DD = ps.tile([128, 5 * R], f16, tag="dd")  # [d22|d11|d33|c1|c2]
                nc.vector.tensor_add(DD[:], SQCP[:, 0:5 * R],
                                     SQCP[:, 5 * R:10 * R])
                nc.vector.tensor_add(DD[:], DD[:], SQCP[:, 10 * R:15 * R])

                LNIN = ps.tile([128, 5 * R], f16, tag="lnin")
                MT = ps.tile([128, 2 * R], f16, tag="mt")   # [m1 | m2]
                DDv = DD[:].rearrange("p (g l) -> p g l", g=5)
                nc.vector.tensor_mul(MT[:].rearrange("p (g l) -> p g l", g=2),
                                     DDv[:, 0:3:2],
                                     DDv[:, 1:2].broadcast_to([128, 2, R]))
                PSQ = ps.tile([128, 2 * R], f16, tag="psq")  # [c1^2 | c2^2]
                nc.vector.tensor_mul(PSQ[:], DD[:, 3 * R:5 * R],
                                     DD[:, 3 * R:5 * R])
                SG = ps.tile([128, 2 * R], f16, tag="sg")
                nc.vector.tensor_scalar(SG[:], DD[:, 3 * R:5 * R], 0.0, None,
                                        op0=Alu.is_ge)
                nc.vector.tensor_scalar(SG[:], SG[:], 2.0, -1.0,
                                        op0=Alu.mult, op1=Alu.add)
                nc.scalar.activation(LNIN[:, R:3 * R], DD[:, 3 * R:5 * R],
                                     Act.Abs)
                nc.vector.tensor_scalar_max(LNIN[:, 0:R], DD[:, R:2 * R],
                                            6e-8)
                LNO = ps.tile([128, 5 * R], f32, tag="lno")
                nc.vector.tensor_sub(LNIN[:, 3 * R:5 * R], MT[:], PSQ[:])
                if CLAMPS:
                    nc.vector.tensor_scalar_max(LNIN[:, R:3 * R],
                                                LNIN[:, R:3 * R], 1e-6)
                    nc.vector.tensor_scalar_max(LNIN[:, 3 * R:5 * R],
                                                LNIN[:, 3 * R:5 * R], 6e-8)
                tbl_order(nc.scalar.activation(LNO[:], LNIN[:], Act.Ln))
                st.append((P, SG, LNO))

            tails = []
            for c in range(CH):
                P, SG, LNO = st[c]
                L = ps.tile([128, 2 * R], f32, tag="l")
                nc.vector.scalar_tensor_tensor(
                    L[:], LNO[:, 3 * R:5 * R], -0.5, LNO[:, R:3 * R],
                    op0=Alu.mult, op1=Alu.add)
                AV = ps.tile([128, 2 * R], f16, tag="av")
                nc.gpsimd.tensor_mul(AV[:], SG[:], P[:, 4 * R:6 * R])
                T3 = ps.tile([128, 3 * R], f16, tag="t3")   # [f1 | phi1 | phi2]
                tbl_order(nc.scalar.activation(T3[:, 0:R], LNO[:, 0:R],
                                                Act.Exp, scale=0.5))
                TH = ps.tile([128, 2 * R], f16, tag="th")
                tbl_order(nc.scalar.activation(TH[:], L[:], Act.Tanh,
                                               scale=0.5))
                tails.append((L, AV, TH, T3))

            # bond leg (needs only exp's f1): fills the DVE idle window
            # while tanh/arctan + their table loads run on ACT
            zcs = []
            for c in range(CH):
                P, SG, LNO = st[c]
                L, AV, TH, T3 = tails[c]
                ZC = ps.tile([128, 3 * R], f16, tag="zc")
                WB = ps.tile([128, R], f16, tag="wb")
                nc.vector.tensor_mul(WB[:], T3[:, 0:R], P[:, 0:R])
                nc.vector.tensor_sub(WB[:], WB[:], P[:, 3 * R:4 * R])
                nc.vector.tensor_mul(WB[:], WB[:], WB[:])
                nc.vector.tensor_tensor(ZC[:, 0:R], WB[:], P[:, 6 * R:7 * R],
                                        op=Alu.min)
                zcs.append(ZC)

            for c in range(CH):
                b0 = c * bc
                P, SG, LNO = st[c]
                L, AV, TH, T3 = tails[c]
                ZC = zcs[c]
                tbl_order(nc.scalar.activation(T3[:, R:3 * R], TH[:],
                                               Act.Arctan))
                nc.vector.tensor_scalar_add(T3[:, R:3 * R], T3[:, R:3 * R],
                                            PI4)
                W = ps.tile([128, 2 * R], f16, tag="w")
                nc.vector.tensor_mul(W[:], T3[:, R:3 * R], P[:, R:3 * R])
                U = ps.tile([128, 2 * R], f16, tag="u")
                nc.vector.tensor_sub(U[:], W[:], AV[:])
                nc.vector.tensor_mul(U[:], U[:], U[:])
                nc.vector.tensor_tensor(ZC[:, R:3 * R], U[:],
                                        P[:, 7 * R:9 * R], op=Alu.min)
                E = ps.tile([128, R], f16, tag="e")
                nc.vector.tensor_add(E[:], ZC[:, 0:R], ZC[:, R:2 * R])
                nc.vector.tensor_add(E[:], E[:], ZC[:, 2 * R:3 * R])
                nc.sync.dma_start(
                    O_t[b0:b0 + bc].rearrange("b c (k l) -> b c k l", k=KC),
                    E[:])

    return nc


def _strip_auto_act_loads(nc):
    """Drop the table loads Bacc's insert_act_table_loads added: its
    first-match set choice ping-pongs between {ln}/{exp}/{arctan} sets.
    Our two manual loads (ln+exp set, tanh+arctan set) cover every
    activation in program order.  The pass runs after semaphore
    generation, so its loads carry no sync info and are safe to remove."""
    from concourse import mybir
    manual = getattr(nc, "_manual_act_loads", set())
    removed = 0
    for f in nc.m.functions:
        for blk in f.blocks:
            keep = []
            for inst in blk.instructions:
                if (isinstance(inst, mybir.InstLoadActFuncSet)
                        and inst.name not in manual):
                    si = inst.sync_info
                    if si is not None and (len(si.on_wait) or len(si.on_update)):
                        keep.append(inst)  # has sync; leave it alone
                        continue
                    removed += 1
                    continue
                keep.append(inst)
            blk.instructions[:] = keep
    return removed


def _get_program():
    if "nc" not in _PROGRAM_CACHE:
        nc = _build_program()
        nc.finalize()   # Bacc: register allocation / DCE / wait legalization
        if bool(int(os.environ.get("BLC_STRIP_LOADS", "0"))):
            _strip_auto_act_loads(nc)
        _PROGRAM_CACHE["nc"] = nc
    return _PROGRAM_CACHE["nc"]


def _host_prep(atom_description, coords, mean, std, weight):
    ad = np.asarray(atom_description)
    coords = np.asarray(coords, dtype=np.float32)
    b, ch, rs, rn, an = (ad[:, i] for i in range(5))
    valid = (b >= 0) & (b < NB) & (ch >= 0) & (ch < MC) & (rs >= 0) & (rs < MR)

    def scat3(mask):
        A = np.full((NB, MC, MR, 3), PAD, np.float32)
        m = mask & valid
        A[b[m], ch[m], rs[m]] = coords[m]
        return A

    Narr, CAarr, Carr = scat3(an == 0), scat3(an == 1), scat3(an == 2)
    seq = np.full((NB, MC, MR), PAD_I, np.int64)
    m = (an == 1) & valid
    seq[b[m], ch[m], rs[m]] = rn[m]

    todo = ((Narr[:, :, 1:, 0] != PAD) & (Carr[:, :, :-1, 0] != PAD)
            & (CAarr[:, :, 1:, 0] != PAD) & (CAarr[:, :, :-1, 0] != PAD)
            & (seq[:, :, 1:] != PAD_I) & (seq[:, :, :-1] != PAD_I))
    sidx = np.clip(np.where(todo, seq[:, :, 1:], 0), 0, 19)

    w0 = float(np.asarray(weight).reshape(-1)[0])
    s_w = 1.0 - np.tanh(-w0)
    sqw = np.sqrt(s_w)
    mu = np.asarray(mean, np.float64)
    sd = np.asarray(std, np.float64)
    qd = 1.0 / (sd * np.sqrt(2.0))
    Q = qd * sqw
    tab = np.empty((20, 9))
    tab[:, 0] = (1.0 / SC) * Q[:, 0]            # QB
    tab[:, 1] = Q[:, 1]                         # QS1
    tab[:, 2] = -Q[:, 2]                        # QS2
    tab[:, 3] = mu[:, 0] * Q[:, 0]              # MU0*Q0
    tab[:, 4] = (np.pi / 2 - mu[:, 1]) * Q[:, 1]  # MQ1
    tab[:, 5] = (np.pi / 2 - mu[:, 2]) * Q[:, 2]  # MQ2
    tab[:, 6:9] = s_w * np.maximum(np.log(CL * qd), 0.0)  # CAP
    tab = tab.astype(np.float32)

    params = np.zeros((NB, MC, MR, 9), np.float32)
    params[:, :, 1:, :] = tab[sidx] * todo[..., None].astype(np.float32)
    pblk = np.ascontiguousarray(
        params.reshape(NB, MC, KC, R, 9).transpose(0, 1, 2, 4, 3)
    ).astype(np.float16)

    G = np.zeros((NB, MC, MR + 1, 9), np.float32)
    G[:, :, 1:, 0:3] = np.where(Narr == PAD, 0.0, Narr) * SC
    G[:, :, 1:, 3:6] = np.where(CAarr == PAD, 0.0, CAarr) * SC
    G[:, :, 1:, 6:9] = np.where(Carr == PAD, 0.0, Carr) * SC
    # blocked plane-contiguous with halo: GB[b,c,k,p,l] = G[b,c,k*R+l,p]
    GB = np.empty((NB, MC, KC, 9, S), np.float32)
    for k in range(KC):
        GB[:, :, k] = G[:, :, k * R:k * R + S, :].transpose(0, 1, 3, 2)
    return GB, pblk


def _install_ntff_hook():
    """The agent image's antenv lacks axon_hooks; synthesize it so
    trace=True can reach the terminal's NRT profiler (dev-only path)."""
    import sys, types
    if "antenv.axon_hooks" in sys.modules:
        return True
    try:
        import antenv
        mod = types.ModuleType("antenv.axon_hooks")
        mod._hook = None

        def set_axon_ntff_profile_hook(h):
            mod._hook = h

        def get_axon_ntff_profile_hook():
            return mod._hook

        mod.set_axon_ntff_profile_hook = set_axon_ntff_profile_hook
        mod.get_axon_ntff_profile_hook = get_axon_ntff_profile_hook
        sys.modules["antenv.axon_hooks"] = mod
        antenv.axon_hooks = mod
        from trn_agent_boot.trn_boot import _ntff_profile_via_ctypes
        mod._hook = _ntff_profile_via_ctypes("/opt/axon/libaxon_pjrt.so")
        return True
    except Exception as e:  # pragma: no cover - profiling is best-effort
        print(f"ntff hook install failed: {e}")
        return False


def kernel(**inputs):
    global LAST_RESULT
    from concourse.bass_utils import run_bass_kernel_spmd
    if TRACE:
        _install_ntff_hook()

    G, pblk = _host_prep(
        inputs["atom_description"], inputs["coords"],
        inputs["mean"], inputs["std"], inputs["weight"])

    nc = _get_program()
    in_maps = [
        {"cx": np.ascontiguousarray(G[i * BPC:(i + 1) * BPC]),
         "pr": np.ascontiguousarray(pblk[i * BPC:(i + 1) * BPC])}
        for i in range(NCORES)
    ]
    res = run_bass_kernel_spmd(nc, in_maps, list(range(NCORES)), trace=TRACE)
    LAST_RESULT = res
    e = np.concatenate([res.results[i]["out"] for i in range(NCORES)], axis=0)
    e = e.astype(np.float32).reshape(NB, MC, MR)
    out = np.repeat(e[..., None], NALT, axis=-1)
    return np.ascontiguousarray(out.astype(np.float32))


# revision 25
# speedup vs baseline: 1.1050x; 1.0087x over previous
"""Trainium2 Bass kernel for nn_BondLenConstrain (v2, fp16 pipeline).

Contract: kernel(**inputs) takes the FULL (unsharded) inputs of
reference.setup_inputs() and returns the full [64, 4, 2048, 2] float32
resiEnergy tensor.  Data-parallel over the batch axis across 8 NeuronCores
(8 batches per core).

Host (numpy, indexing only): scatter atoms into dense residue grids,
build the `todo` mask, gather the tiny per-residue-type tables into
per-pair fp16 coefficient planes (masked pairs get all-zero coefficients
-> device returns exactly 0), and lay out coords (f32, prescaled by 1/16)
in a plane-contiguous blocked layout with a one-slot halo.

Device math per residue pair (r-1, r), fp16 unless noted:
    v2 = CA_r - N_r, v1 = C_{r-1} - N_r, v3 = CA_{r-1} - C_{r-1}
    (subtractions read f32 coords, write fp16 - avoids cancellation loss)
    d11,d22,d33,c1,c2 via one self-mul + one cross-mul + two segmented adds
    ln pass (f32 out) over [d11 | s1^2 | s2^2 | |c1| | |c2|]
    f1 = exp(0.5 ln d11)
    L = ln|c| - 0.5 ln(s^2)  (= ln t, t = |c|/s)   [f32]
    phi = pi/4 + arctan(tanh(L/2))    <- Gudermannian identity replaces the
        sign/exp range-reduction dance: arctan(t) = pi/4 + arctan(tanh(ln(t)/2))
    U = [f1*QB - mu0*Q0 | phi*(+-Q) - sign(c)*(pi/2-mu)*Q]   (x-> -sx, squared)
    E = sum_d min(U^2, CAP)
ACT function tables: free-running per-chunk order (ln -> exp -> tanh ->
arctan); square/sign/abs ride every set.  A dummy Ln hides the first
table load inside the DMA fill.  Forcing globally grouped table order
measured slower (serializes the per-chunk tails past the last front).
Sign is computed on DVE (two 4x tensor_scalar ops) so no const AP or
preamble barrier is needed.  The degenerate-geometry clamps are off by
default (BLC_CLAMPS=0): s^2<=0 / c=0 produce NaN/-inf which flow through
tanh/arctan to the capped branch (DVE min returns the non-NaN operand),
matching the reference's capped score exactly.
"""

import os
import numpy as np

PAD = -999.0
PAD_I = -999
NB, MC, MR = 64, 4, 2048
NALT = 2
NCORES = 8
BPC = NB // NCORES            # batches per core
CH = int(os.environ.get("BLC_CHUNKS", "2"))  # pipeline chunks per core
KC = 4 * CH                   # blocks per (batch, chain) across full chain
R = MR // KC                  # residues (pairs) per partition
S = R + 1                     # coord slots per plane (halo)
EPS = 1e-12
CL = 1.0 / (EPS * np.sqrt(np.pi))
SC = 1.0 / 16.0               # coord prescale (fp16 range safety)
CLAMPS = bool(int(os.environ.get("BLC_CLAMPS", "0")))

_PROGRAM_CACHE = {}
LAST_RESULT = None            # BassKernelResults of the last run (for test.py)
TRACE = bool(int(os.environ.get("BLC_TRACE", "0")))


def _build_program():
    import concourse.bass as bass
    import concourse.tile as tile
    from concourse import bacc, mybir
    from concourse.bass import _add_dep_helper
    f16 = mybir.dt.float16
    f32 = mybir.dt.float32
    Alu = mybir.AluOpType
    Act = mybir.ActivationFunctionType

    nc = bacc.Bacc("TRN2", target_bir_lowering=False, debug=False)

    X_t = nc.declare_dram_parameter("cx", [BPC, MC, KC, 9, S], f32,
                                    isOutput=False)
    P_t = nc.declare_dram_parameter("pr", [BPC, MC, KC, 9, R], f16,
                                    isOutput=False)
    O_t = nc.declare_dram_parameter("out", [BPC, MC, MR], f16, isOutput=True)


    bc = BPC // CH            # batches per chunk
    bufs = min(CH, 2)
    PI4 = float(np.pi / 4)

    with tile.TileContext(nc) as tc:
        with (
            tc.tile_pool(name="px", bufs=bufs) as px,
            tc.tile_pool(name="ps", bufs=bufs) as ps,
        ):
            # ---- DMA loads for all chunks up front (one sync ring) -------
            # coords first (they gate the fronts); coefficient planes are
            # only needed by the tails, so they stream last
            loads = []
            for c in range(CH):
                b0 = c * bc
                X = px.tile([128, 9 * S], f32, tag="x")
                P = px.tile([128, 9 * R], f16, tag="p")
                # split the coord load: N+CA planes first so the v2
                # subtraction can start while C planes still stream
                nc.sync.dma_start(X[:, 0:6 * S], X_t[b0:b0 + bc, :, :, 0:6])
                nc.sync.dma_start(X[:, 6 * S:9 * S],
                                  X_t[b0:b0 + bc, :, :, 6:9])
                loads.append((X, P))
            for c in range(CH):
                b0 = c * bc
                X, P = loads[c]
                nc.sync.dma_start(P[:], P_t[b0:b0 + bc])

            # dummy Ln on a tiny scratch tile: preloads the {ln} table
            # under the DMA fill (input value is irrelevant)
            dscr = ps.tile([128, 1], f32, tag="dummy")
            nc.vector.memset(dscr[:], 1.0)
            dummy_ln = nc.scalar.activation(dscr[:], dscr[:], Act.Ln)
            tblchain = [dummy_ln]

            def tbl_order(bi):
                _add_dep_helper(bi.ins, tblchain[-1].ins, sync=False,
                                reason="act-table-order")
                tblchain.append(bi)
                return bi

            # ---- phase 1a: difference vectors + products, per chunk ------
            # (squares ride ACT while DVE moves on to the next chunk's subs)
            geo = []
            for c in range(CH):
                X, P = loads[c]
                Xv = X[:].rearrange("p (n l) -> p n l", n=9)
                # c-major: V[p, c, v, l], v in {v2, v1, v3} -> the three
                # per-coordinate SQCP blocks become contiguous 5R adds
                V = px.tile([128, 9 * R], f16, tag="v")
                Vv = V[:].rearrange("p (cc v l) -> p cc v l", cc=3, v=3)
                nc.vector.tensor_sub(Vv[:, :, 0], Xv[:, 3:6, 1:S],
                                     Xv[:, 0:3, 1:S])
                nc.vector.tensor_sub(Vv[:, :, 1], Xv[:, 6:9, 0:R],
                                     Xv[:, 0:3, 1:S])
                nc.vector.tensor_sub(Vv[:, :, 2], Xv[:, 3:6, 0:R],
                                     Xv[:, 6:9, 0:R])
                SQCP = px.tile([128, 15 * R], f16, tag="sqcp")
                Qv = SQCP[:].rearrange("p (cc g l) -> p cc g l", cc=3, g=5)
                nc.vector.tensor_mul(Qv[:, :, 3:5], Vv[:, :, 1:3],
                                     Vv[:, :, 0:2])
                nc.scalar.square(Qv[:, :, 0:3], Vv[:])
                geo.append((V, SQCP))

            # ---- phase 1b: reductions down to the ln, per chunk ----------
            st = []
            for c in range(CH):
                X, P = loads[c]
                V, SQCP = geo[c]
                DD = ps.tile([128, 5 * R], f16, tag="dd")  # [d22|d11|d33|c1|c2]
                nc.vector.tensor_add(DD[:], SQCP[:, 0:5 * R],
                                     SQCP[:, 5 * R:10 * R])
                nc.vector.tensor_add(DD[:], DD[:], SQCP[:, 10 * R:15 * R])

                LNIN = ps.tile([128, 5 * R], f16, tag="lnin")
                MT = ps.tile([128, 2 * R], f16, tag="mt")   # [m1 | m2]
                DDv = DD[:].rearrange("p (g l) -> p g l", g=5)
                nc.vector.tensor_mul(MT[:].rearrange("p (g l) -> p g l", g=2),
                                     DDv[:, 0:3:2],
                                     DDv[:, 1:2].broadcast_to([128, 2, R]))
                PSQ = ps.tile([128, 2 * R], f16, tag="psq")  # [c1^2 | c2^2]
                nc.vector.tensor_mul(PSQ[:], DD[:, 3 * R:5 * R],
                                     DD[:, 3 * R:5 * R])
                SG = ps.tile([128, 2 * R], f16, tag="sg")
                nc.vector.tensor_scalar(SG[:], DD[:, 3 * R:5 * R], 0.0, None,
                                        op0=Alu.is_ge)
                nc.vector.tensor_scalar(SG[:], SG[:], 2.0, -1.0,
                                        op0=Alu.mult, op1=Alu.add)
                nc.scalar.activation(LNIN[:, R:3 * R], DD[:, 3 * R:5 * R],
                                     Act.Abs)
                nc.vector.tensor_scalar_max(LNIN[:, 0:R], DD[:, R:2 * R],
                                            6e-8)
                LNO = ps.tile([128, 5 * R], f32, tag="lno")
                if CLAMPS:
                    nc.vector.tensor_scalar_max(LNIN[:, R:3 * R],
                                                LNIN[:, R:3 * R], 1e-6)
                if c == CH - 1:
                    # last chunk gates the whole tail chain: let the
                    # d11+|c| part of the ln start before s^2 lands
                    tbl_order(nc.scalar.activation(LNO[:, 0:3 * R],
                                                   LNIN[:, 0:3 * R], Act.Ln))
                    nc.vector.tensor_sub(LNIN[:, 3 * R:5 * R], MT[:], PSQ[:])
                    if CLAMPS:
                        nc.vector.tensor_scalar_max(LNIN[:, 3 * R:5 * R],
                                                    LNIN[:, 3 * R:5 * R],
                                                    6e-8)
                    tbl_order(nc.scalar.activation(LNO[:, 3 * R:5 * R],
                                                   LNIN[:, 3 * R:5 * R],
                                                   Act.Ln))
                else:
                    nc.vector.tensor_sub(LNIN[:, 3 * R:5 * R], MT[:], PSQ[:])
                    if CLAMPS:
                        nc.vector.tensor_scalar_max(LNIN[:, 3 * R:5 * R],
                                                    LNIN[:, 3 * R:5 * R],
                                                    6e-8)
                    tbl_order(nc.scalar.activation(LNO[:], LNIN[:], Act.Ln))
                st.append((P, SG, LNO))

            tails = []
            for c in range(CH):
                P, SG, LNO = st[c]
                L = ps.tile([128, 2 * R], f32, tag="l")
                nc.vector.scalar_tensor_tensor(
                    L[:], LNO[:, 3 * R:5 * R], -0.5, LNO[:, R:3 * R],
                    op0=Alu.mult, op1=Alu.add)
                AV = ps.tile([128, 2 * R], f16, tag="av")
                nc.gpsimd.tensor_mul(AV[:], SG[:], P[:, 4 * R:6 * R])
                T3 = ps.tile([128, 3 * R], f16, tag="t3")   # [f1 | phi1 | phi2]
                tbl_order(nc.scalar.activation(T3[:, 0:R], LNO[:, 0:R],
                                                Act.Exp, scale=0.5))
                TH = ps.tile([128, 2 * R], f16, tag="th")
                tbl_order(nc.scalar.activation(TH[:], L[:], Act.Tanh,
                                               scale=0.5))
                tails.append((L, AV, TH, T3))

            # bond leg (needs only exp's f1): fills the DVE idle window
            # while tanh/arctan + their table loads run on ACT
            zcs = []
            for c in range(CH):
                P, SG, LNO = st[c]
                L, AV, TH, T3 = tails[c]
                ZC = ps.tile([128, 3 * R], f16, tag="zc")
                WB = ps.tile([128, R], f16, tag="wb")
                nc.vector.tensor_mul(WB[:], T3[:, 0:R], P[:, 0:R])
                nc.vector.tensor_sub(WB[:], WB[:], P[:, 3 * R:4 * R])
                nc.vector.tensor_mul(WB[:], WB[:], WB[:])
                nc.vector.tensor_tensor(ZC[:, 0:R], WB[:], P[:, 6 * R:7 * R],
                                        op=Alu.min)
                zcs.append(ZC)

            for c in range(CH):
                b0 = c * bc
                P, SG, LNO = st[c]
                L, AV, TH, T3 = tails[c]
                ZC = zcs[c]
                tbl_order(nc.scalar.activation(T3[:, R:3 * R], TH[:],
                                               Act.Arctan))
                nc.vector.tensor_scalar_add(T3[:, R:3 * R], T3[:, R:3 * R],
                                            PI4)
                W = ps.tile([128, 2 * R], f16, tag="w")
                nc.vector.tensor_mul(W[:], T3[:, R:3 * R], P[:, R:3 * R])
                U = ps.tile([128, 2 * R], f16, tag="u")
                nc.vector.tensor_sub(U[:], W[:], AV[:])
                nc.vector.tensor_mul(U[:], U[:], U[:])
                nc.vector.tensor_tensor(ZC[:, R:3 * R], U[:],
                                        P[:, 7 * R:9 * R], op=Alu.min)
                E = ps.tile([128, R], f16, tag="e")
                nc.vector.tensor_add(E[:], ZC[:, 0:R], ZC[:, R:2 * R])
                nc.vector.tensor_add(E[:], E[:], ZC[:, 2 * R:3 * R])
                nc.sync.dma_start(
                    O_t[b0:b0 + bc].rearrange("b c (k l) -> b c k l", k=KC),
                    E[:])

    return nc


def _strip_auto_act_loads(nc):
    """Drop the table loads Bacc's insert_act_table_loads added: its
    first-match set choice ping-pongs between {ln}/{exp}/{arctan} sets.
    Our two manual loads (ln+exp set, tanh+arctan set) cover every
    activation in program order.  The pass runs after semaphore
    generation, so its loads carry no sync info and are safe to remove."""
    from concourse import mybir
    manual = getattr(nc, "_manual_act_loads", set())
    removed = 0
    for f in nc.m.functions:
        for blk in f.blocks:
            keep = []
            for inst in blk.instructions:
                if (isinstance(inst, mybir.InstLoadActFuncSet)
                        and inst.name not in manual):
                    si = inst.sync_info
                    if si is not None and (len(si.on_wait) or len(si.on_update)):
                        keep.append(inst)  # has sync; leave it alone
                        continue
                    removed += 1
                    continue
                keep.append(inst)
            blk.instructions[:] = keep
    return removed


def _get_program():
    if "nc" not in _PROGRAM_CACHE:
        nc = _build_program()
        nc.finalize()   # Bacc: register allocation / DCE / wait legalization
        if bool(int(os.environ.get("BLC_STRIP_LOADS", "0"))):
            _strip_auto_act_loads(nc)
        _PROGRAM_CACHE["nc"] = nc
    return _PROGRAM_CACHE["nc"]


def _host_prep(atom_description, coords, mean, std, weight):
    ad = np.asarray(atom_description)
    coords = np.asarray(coords, dtype=np.float32)
    b, ch, rs, rn, an = (ad[:, i] for i in range(5))
    valid = (b >= 0) & (b < NB) & (ch >= 0) & (ch < MC) & (rs >= 0) & (rs < MR)

    def scat3(mask):
        A = np.full((NB, MC, MR, 3), PAD, np.float32)
        m = mask & valid
        A[b[m], ch[m], rs[m]] = coords[m]
        return A

    Narr, CAarr, Carr = scat3(an == 0), scat3(an == 1), scat3(an == 2)
    seq = np.full((NB, MC, MR), PAD_I, np.int64)
    m = (an == 1) & valid
    seq[b[m], ch[m], rs[m]] = rn[m]

    todo = ((Narr[:, :, 1:, 0] != PAD) & (Carr[:, :, :-1, 0] != PAD)
            & (CAarr[:, :, 1:, 0] != PAD) & (CAarr[:, :, :-1, 0] != PAD)
            & (seq[:, :, 1:] != PAD_I) & (seq[:, :, :-1] != PAD_I))
    sidx = np.clip(np.where(todo, seq[:, :, 1:], 0), 0, 19)

    w0 = float(np.asarray(weight).reshape(-1)[0])
    s_w = 1.0 - np.tanh(-w0)
    sqw = np.sqrt(s_w)
    mu = np.asarray(mean, np.float64)
    sd = np.asarray(std, np.float64)
    qd = 1.0 / (sd * np.sqrt(2.0))
    Q = qd * sqw
    tab = np.empty((20, 9))
    tab[:, 0] = (1.0 / SC) * Q[:, 0]            # QB
    tab[:, 1] = Q[:, 1]                         # QS1
    tab[:, 2] = -Q[:, 2]                        # QS2
    tab[:, 3] = mu[:, 0] * Q[:, 0]              # MU0*Q0
    tab[:, 4] = (np.pi / 2 - mu[:, 1]) * Q[:, 1]  # MQ1
    tab[:, 5] = (np.pi / 2 - mu[:, 2]) * Q[:, 2]  # MQ2
    tab[:, 6:9] = s_w * np.maximum(np.log(CL * qd), 0.0)  # CAP
    tab = tab.astype(np.float32)

    params = np.zeros((NB, MC, MR, 9), np.float32)
    params[:, :, 1:, :] = tab[sidx] * todo[..., None].astype(np.float32)
    pblk = np.ascontiguousarray(
        params.reshape(NB, MC, KC, R, 9).transpose(0, 1, 2, 4, 3)
    ).astype(np.float16)

    G = np.zeros((NB, MC, MR + 1, 9), np.float32)
    G[:, :, 1:, 0:3] = np.where(Narr == PAD, 0.0, Narr) * SC
    G[:, :, 1:, 3:6] = np.where(CAarr == PAD, 0.0, CAarr) * SC
    G[:, :, 1:, 6:9] = np.where(Carr == PAD, 0.0, Carr) * SC
    # blocked plane-contiguous with halo: GB[b,c,k,p,l] = G[b,c,k*R+l,p]
    GB = np.empty((NB, MC, KC, 9, S), np.float32)
    for k in range(KC):
        GB[:, :, k] = G[:, :, k * R:k * R + S, :].transpose(0, 1, 3, 2)
    return GB, pblk


def _install_ntff_hook():
    """The agent image's antenv lacks axon_hooks; synthesize it so
    trace=True can reach the terminal's NRT profiler (dev-only path)."""
    import sys, types
    if "antenv.axon_hooks" in sys.modules:
        return True
    try:
        import antenv
        mod = types.ModuleType("antenv.axon_hooks")
        mod._hook = None

        def set_axon_ntff_profile_hook(h):
            mod._hook = h

        def get_axon_ntff_profile_hook():
            return mod._hook

        mod.set_axon_ntff_profile_hook = set_axon_ntff_profile_hook
        mod.get_axon_ntff_profile_hook = get_axon_ntff_profile_hook
        sys.modules["antenv.axon_hooks"] = mod
        antenv.axon_hooks = mod
        from trn_agent_boot.trn_boot import _ntff_profile_via_ctypes
        mod._hook = _ntff_profile_via_ctypes("/opt/axon/libaxon_pjrt.so")
        return True
    except Exception as e:  # pragma: no cover - profiling is best-effort
        print(f"ntff hook install failed: {e}")
        return False


def kernel(**inputs):
    global LAST_RESULT
    from concourse.bass_utils import run_bass_kernel_spmd
    if TRACE:
        _install_ntff_hook()

    G, pblk = _host_prep(
        inputs["atom_description"], inputs["coords"],
        inputs["mean"], inputs["std"], inputs["weight"])

    nc = _get_program()
    in_maps = [
        {"cx": np.ascontiguousarray(G[i * BPC:(i + 1) * BPC]),
         "pr": np.ascontiguousarray(pblk[i * BPC:(i + 1) * BPC])}
        for i in range(NCORES)
    ]
    res = run_bass_kernel_spmd(nc, in_maps, list(range(NCORES)), trace=TRACE)
    LAST_RESULT = res
    e = np.concatenate([res.results[i]["out"] for i in range(NCORES)], axis=0)
    e = e.astype(np.float32).reshape(NB, MC, MR)
    out = np.repeat(e[..., None], NALT, axis=-1)
    return np.ascontiguousarray(out.astype(np.float32))


# revision 26
# speedup vs baseline: 1.1376x; 1.0294x over previous
"""Trainium2 Bass kernel for nn_BondLenConstrain (v2, fp16 pipeline).

Contract: kernel(**inputs) takes the FULL (unsharded) inputs of
reference.setup_inputs() and returns the full [64, 4, 2048, 2] float32
resiEnergy tensor.  Data-parallel over the batch axis across 8 NeuronCores
(8 batches per core).

Host (numpy, indexing only): scatter atoms into dense residue grids,
build the `todo` mask, gather the tiny per-residue-type tables into
per-pair fp16 coefficient planes (masked pairs get all-zero coefficients
-> device returns exactly 0), and lay out coords (f32, prescaled by 1/16)
in a plane-contiguous blocked layout with a one-slot halo.

Device math per residue pair (r-1, r), fp16 unless noted:
    v2 = CA_r - N_r, v1 = C_{r-1} - N_r, v3 = CA_{r-1} - C_{r-1}
    (subtractions read f32 coords, write fp16 - avoids cancellation loss)
    d11,d22,d33,c1,c2 via one self-mul + one cross-mul + two segmented adds
    ln pass (f32 out) over [d11 | s1^2 | s2^2 | |c1| | |c2|]
    f1 = exp(0.5 ln d11)
    L = ln|c| - 0.5 ln(s^2)  (= ln t, t = |c|/s)   [f32]
    phi = pi/4 + arctan(tanh(L/2))    <- Gudermannian identity replaces the
        sign/exp range-reduction dance: arctan(t) = pi/4 + arctan(tanh(ln(t)/2))
    U = [f1*QB - mu0*Q0 | phi*(+-Q) - sign(c)*(pi/2-mu)*Q]   (x-> -sx, squared)
    E = sum_d min(U^2, CAP)
ACT function tables: free-running per-chunk order (ln -> exp -> tanh ->
arctan); square/sign/abs ride every set.  A dummy Ln hides the first
table load inside the DMA fill.  Forcing globally grouped table order
measured slower (serializes the per-chunk tails past the last front).
Sign is computed on DVE (two 4x tensor_scalar ops) so no const AP or
preamble barrier is needed.  The degenerate-geometry clamps are off by
default (BLC_CLAMPS=0): s^2<=0 / c=0 produce NaN/-inf which flow through
tanh/arctan to the capped branch (DVE min returns the non-NaN operand),
matching the reference's capped score exactly.
"""

import os
import numpy as np

PAD = -999.0
PAD_I = -999
NB, MC, MR = 64, 4, 2048
NALT = 2
NCORES = 8
BPC = NB // NCORES            # batches per core
CH = int(os.environ.get("BLC_CHUNKS", "2"))  # pipeline chunks per core
KC = 4 * CH                   # blocks per (batch, chain) across full chain
R = MR // KC                  # residues (pairs) per partition
S = R + 1                     # coord slots per plane (halo)
EPS = 1e-12
CL = 1.0 / (EPS * np.sqrt(np.pi))
SC = 1.0 / 16.0               # coord prescale (fp16 range safety)
CLAMPS = bool(int(os.environ.get("BLC_CLAMPS", "0")))

_PROGRAM_CACHE = {}
LAST_RESULT = None            # BassKernelResults of the last run (for test.py)
TRACE = bool(int(os.environ.get("BLC_TRACE", "0")))


def _build_program():
    import concourse.bass as bass
    import concourse.tile as tile
    from concourse import bacc, mybir
    from concourse.bass import _add_dep_helper
    f16 = mybir.dt.float16
    f32 = mybir.dt.float32
    Alu = mybir.AluOpType
    Act = mybir.ActivationFunctionType

    nc = bacc.Bacc("TRN2", target_bir_lowering=False, debug=False)

    X_t = nc.declare_dram_parameter("cx", [BPC, MC, KC, 9, S], f32,
                                    isOutput=False)
    P_t = nc.declare_dram_parameter("pr", [BPC, MC, KC, 9, R], f16,
                                    isOutput=False)
    O_t = nc.declare_dram_parameter("out", [BPC, MC, MR], f16, isOutput=True)


    bc = BPC // CH            # batches per chunk
    bufs = min(CH, 2)
    PI4 = float(np.pi / 4)

    with tile.TileContext(nc) as tc:
        with (
            tc.tile_pool(name="px", bufs=bufs) as px,
            tc.tile_pool(name="ps", bufs=bufs) as ps,
        ):
            # ---- DMA loads for all chunks up front (one sync ring) -------
            # coords first (they gate the fronts); coefficient planes are
            # only needed by the tails, so they stream last
            loads = []
            for c in range(CH):
                b0 = c * bc
                X = px.tile([128, 9 * S], f32, tag="x")
                P = px.tile([128, 9 * R], f16, tag="p")
                # split the coord load: N+CA planes first so the v2
                # subtraction can start while C planes still stream
                nc.sync.dma_start(X[:, 0:6 * S], X_t[b0:b0 + bc, :, :, 0:6])
                nc.sync.dma_start(X[:, 6 * S:9 * S],
                                  X_t[b0:b0 + bc, :, :, 6:9])
                loads.append((X, P))
            for c in range(CH):
                b0 = c * bc
                X, P = loads[c]
                nc.sync.dma_start(P[:], P_t[b0:b0 + bc])

            # dummy Ln on a tiny scratch tile: preloads the {ln} table
            # under the DMA fill (input value is irrelevant)
            dscr = ps.tile([128, 1], f32, tag="dummy")
            nc.vector.memset(dscr[:], 1.0)
            dummy_ln = nc.scalar.activation(dscr[:], dscr[:], Act.Ln)
            tblchain = [dummy_ln]

            def tbl_order(bi):
                _add_dep_helper(bi.ins, tblchain[-1].ins, sync=False,
                                reason="act-table-order")
                tblchain.append(bi)
                return bi

            # ---- phase 1a: difference vectors + products, per chunk ------
            # (squares ride ACT while DVE moves on to the next chunk's subs)
            geo = []
            for c in range(CH):
                X, P = loads[c]
                Xv = X[:].rearrange("p (n l) -> p n l", n=9)
                # c-major: V[p, c, v, l], v in {v2, v1, v3} -> the three
                # per-coordinate SQCP blocks become contiguous 5R adds
                V = px.tile([128, 9 * R], f16, tag="v")
                Vv = V[:].rearrange("p (cc v l) -> p cc v l", cc=3, v=3)
                nc.vector.tensor_sub(Vv[:, :, 0], Xv[:, 3:6, 1:S],
                                     Xv[:, 0:3, 1:S])
                nc.vector.tensor_sub(Vv[:, :, 1], Xv[:, 6:9, 0:R],
                                     Xv[:, 0:3, 1:S])
                nc.vector.tensor_sub(Vv[:, :, 2], Xv[:, 3:6, 0:R],
                                     Xv[:, 6:9, 0:R])
                SQCP = px.tile([128, 15 * R], f16, tag="sqcp")
                Qv = SQCP[:].rearrange("p (cc g l) -> p cc g l", cc=3, g=5)
                nc.vector.tensor_mul(Qv[:, :, 3:5], Vv[:, :, 1:3],
                                     Vv[:, :, 0:2])
                nc.scalar.square(Qv[:, :, 0:3], Vv[:])
                geo.append((V, SQCP))

            # ---- phase 1b: reductions down to the ln, per chunk ----------
            st = []
            for c in range(CH):
                X, P = loads[c]
                V, SQCP = geo[c]
                DD = ps.tile([128, 5 * R], f16, tag="dd")  # [d22|d11|d33|c1|c2]
                nc.vector.tensor_add(DD[:], SQCP[:, 0:5 * R],
                                     SQCP[:, 5 * R:10 * R])
                nc.vector.tensor_add(DD[:], DD[:], SQCP[:, 10 * R:15 * R])

                LNIN = ps.tile([128, 5 * R], f16, tag="lnin")
                MT = ps.tile([128, 2 * R], f16, tag="mt")   # [m1 | m2]
                DDv = DD[:].rearrange("p (g l) -> p g l", g=5)
                nc.vector.tensor_mul(MT[:].rearrange("p (g l) -> p g l", g=2),
                                     DDv[:, 0:3:2],
                                     DDv[:, 1:2].broadcast_to([128, 2, R]))
                PSQ = ps.tile([128, 2 * R], f16, tag="psq")  # [c1^2 | c2^2]
                nc.vector.tensor_mul(PSQ[:], DD[:, 3 * R:5 * R],
                                     DD[:, 3 * R:5 * R])
                SG = ps.tile([128, 2 * R], f16, tag="sg")
                nc.vector.tensor_scalar(SG[:], DD[:, 3 * R:5 * R], 0.0, None,
                                        op0=Alu.is_ge)
                nc.vector.tensor_scalar(SG[:], SG[:], 2.0, -1.0,
                                        op0=Alu.mult, op1=Alu.add)
                nc.scalar.activation(LNIN[:, R:3 * R], DD[:, 3 * R:5 * R],
                                     Act.Abs)
                nc.vector.tensor_scalar_max(LNIN[:, 0:R], DD[:, R:2 * R],
                                            6e-8)
                LNO = ps.tile([128, 5 * R], f32, tag="lno")
                nc.vector.tensor_sub(LNIN[:, 3 * R:5 * R], MT[:], PSQ[:])
                if CLAMPS:
                    nc.vector.tensor_scalar_max(LNIN[:, R:3 * R],
                                                LNIN[:, R:3 * R], 1e-6)
                    nc.vector.tensor_scalar_max(LNIN[:, 3 * R:5 * R],
                                                LNIN[:, 3 * R:5 * R], 6e-8)
                tbl_order(nc.scalar.activation(LNO[:], LNIN[:], Act.Ln))
                st.append((P, SG, LNO))

            tails = []
            for c in range(CH):
                P, SG, LNO = st[c]
                L = ps.tile([128, 2 * R], f32, tag="l")
                nc.vector.scalar_tensor_tensor(
                    L[:], LNO[:, 3 * R:5 * R], -0.5, LNO[:, R:3 * R],
                    op0=Alu.mult, op1=Alu.add)
                AV = ps.tile([128, 2 * R], f16, tag="av")
                nc.gpsimd.tensor_mul(AV[:], SG[:], P[:, 4 * R:6 * R])
                T3 = ps.tile([128, 3 * R], f16, tag="t3")   # [f1 | phi1 | phi2]
                tbl_order(nc.scalar.activation(T3[:, 0:R], LNO[:, 0:R],
                                                Act.Exp, scale=0.5))
                TH = ps.tile([128, 2 * R], f16, tag="th")
                tbl_order(nc.scalar.activation(TH[:], L[:], Act.Tanh,
                                               scale=0.5))
                tails.append((L, AV, TH, T3))

            # bond leg (needs only exp's f1): fills the DVE idle window
            # while tanh/arctan + their table loads run on ACT
            zcs = []
            for c in range(CH):
                P, SG, LNO = st[c]
                L, AV, TH, T3 = tails[c]
                ZC = ps.tile([128, 3 * R], f16, tag="zc")
                WB = ps.tile([128, R], f16, tag="wb")
                nc.vector.tensor_mul(WB[:], T3[:, 0:R], P[:, 0:R])
                nc.vector.tensor_sub(WB[:], WB[:], P[:, 3 * R:4 * R])
                nc.vector.tensor_mul(WB[:], WB[:], WB[:])
                nc.vector.tensor_tensor(ZC[:, 0:R], WB[:], P[:, 6 * R:7 * R],
                                        op=Alu.min)
                zcs.append(ZC)

            for c in range(CH):
                b0 = c * bc
                P, SG, LNO = st[c]
                L, AV, TH, T3 = tails[c]
                ZC = zcs[c]
                tbl_order(nc.scalar.activation(T3[:, R:3 * R], TH[:],
                                               Act.Arctan))
                nc.vector.tensor_scalar_add(T3[:, R:3 * R], T3[:, R:3 * R],
                                            PI4)
                W = ps.tile([128, 2 * R], f16, tag="w")
                nc.vector.tensor_mul(W[:], T3[:, R:3 * R], P[:, R:3 * R])
                U = ps.tile([128, 2 * R], f16, tag="u")
                nc.vector.tensor_sub(U[:], W[:], AV[:])
                nc.vector.tensor_mul(U[:], U[:], U[:])
                nc.vector.tensor_tensor(ZC[:, R:3 * R], U[:],
                                        P[:, 7 * R:9 * R], op=Alu.min)
                E = ps.tile([128, R], f16, tag="e")
                nc.vector.tensor_add(E[:], ZC[:, 0:R], ZC[:, R:2 * R])
                nc.vector.tensor_add(E[:], E[:], ZC[:, 2 * R:3 * R])
                nc.sync.dma_start(
                    O_t[b0:b0 + bc].rearrange("b c (k l) -> b c k l", k=KC),
                    E[:])

    return nc


def _strip_auto_act_loads(nc):
    """Drop the table loads Bacc's insert_act_table_loads added: its
    first-match set choice ping-pongs between {ln}/{exp}/{arctan} sets.
    Our two manual loads (ln+exp set, tanh+arctan set) cover every
    activation in program order.  The pass runs after semaphore
    generation, so its loads carry no sync info and are safe to remove."""
    from concourse import mybir
    manual = getattr(nc, "_manual_act_loads", set())
    removed = 0
    for f in nc.m.functions:
        for blk in f.blocks:
            keep = []
            for inst in blk.instructions:
                if (isinstance(inst, mybir.InstLoadActFuncSet)
                        and inst.name not in manual):
                    si = inst.sync_info
                    if si is not None and (len(si.on_wait) or len(si.on_update)):
                        keep.append(inst)  # has sync; leave it alone
                        continue
                    removed += 1
                    continue
                keep.append(inst)
            blk.instructions[:] = keep
    return removed


def _get_program():
    if "nc" not in _PROGRAM_CACHE:
        nc = _build_program()
        nc.finalize()   # Bacc: register allocation / DCE / wait legalization
        if bool(int(os.environ.get("BLC_STRIP_LOADS", "0"))):
            _strip_auto_act_loads(nc)
        _PROGRAM_CACHE["nc"] = nc
    return _PROGRAM_CACHE["nc"]


def _host_prep(atom_description, coords, mean, std, weight):
    ad = np.asarray(atom_description)
    coords = np.asarray(coords, dtype=np.float32)
    b, ch, rs, rn, an = (ad[:, i] for i in range(5))
    valid = (b >= 0) & (b < NB) & (ch >= 0) & (ch < MC) & (rs >= 0) & (rs < MR)

    def scat3(mask):
        A = np.full((NB, MC, MR, 3), PAD, np.float32)
        m = mask & valid
        A[b[m], ch[m], rs[m]] = coords[m]
        return A

    Narr, CAarr, Carr = scat3(an == 0), scat3(an == 1), scat3(an == 2)
    seq = np.full((NB, MC, MR), PAD_I, np.int64)
    m = (an == 1) & valid
    seq[b[m], ch[m], rs[m]] = rn[m]

    todo = ((Narr[:, :, 1:, 0] != PAD) & (Carr[:, :, :-1, 0] != PAD)
            & (CAarr[:, :, 1:, 0] != PAD) & (CAarr[:, :, :-1, 0] != PAD)
            & (seq[:, :, 1:] != PAD_I) & (seq[:, :, :-1] != PAD_I))
    sidx = np.clip(np.where(todo, seq[:, :, 1:], 0), 0, 19)

    w0 = float(np.asarray(weight).reshape(-1)[0])
    s_w = 1.0 - np.tanh(-w0)
    sqw = np.sqrt(s_w)
    mu = np.asarray(mean, np.float64)
    sd = np.asarray(std, np.float64)
    qd = 1.0 / (sd * np.sqrt(2.0))
    Q = qd * sqw
    tab = np.empty((20, 9))
    tab[:, 0] = (1.0 / SC) * Q[:, 0]            # QB
    tab[:, 1] = Q[:, 1]                         # QS1
    tab[:, 2] = -Q[:, 2]                        # QS2
    tab[:, 3] = mu[:, 0] * Q[:, 0]              # MU0*Q0
    tab[:, 4] = (np.pi / 2 - mu[:, 1]) * Q[:, 1]  # MQ1
    tab[:, 5] = (np.pi / 2 - mu[:, 2]) * Q[:, 2]  # MQ2
    tab[:, 6:9] = s_w * np.maximum(np.log(CL * qd), 0.0)  # CAP
    tab = tab.astype(np.float32)

    params = np.zeros((NB, MC, MR, 9), np.float32)
    params[:, :, 1:, :] = tab[sidx] * todo[..., None].astype(np.float32)
    pblk = np.ascontiguousarray(
        params.reshape(NB, MC, KC, R, 9).transpose(0, 1, 2, 4, 3)
    ).astype(np.float16)

    G = np.zeros((NB, MC, MR + 1, 9), np.float32)
    G[:, :, 1:, 0:3] = np.where(Narr == PAD, 0.0, Narr) * SC
    G[:, :, 1:, 3:6] = np.where(CAarr == PAD, 0.0, CAarr) * SC
    G[:, :, 1:, 6:9] = np.where(Carr == PAD, 0.0, Carr) * SC
    # blocked plane-contiguous with halo: GB[b,c,k,p,l] = G[b,c,k*R+l,p]
    GB = np.empty((NB, MC, KC, 9, S), np.float32)
    for k in range(KC):
        GB[:, :, k] = G[:, :, k * R:k * R + S, :].transpose(0, 1, 3, 2)
    return GB, pblk


def _install_ntff_hook():
    """The agent image's antenv lacks axon_hooks; synthesize it so
    trace=True can reach the terminal's NRT profiler (dev-only path)."""
    import sys, types
    if "antenv.axon_hooks" in sys.modules:
        return True
    try:
        import antenv
        mod = types.ModuleType("antenv.axon_hooks")
        mod._hook = None

        def set_axon_ntff_profile_hook(h):
            mod._hook = h

        def get_axon_ntff_profile_hook():
            return mod._hook

        mod.set_axon_ntff_profile_hook = set_axon_ntff_profile_hook
        mod.get_axon_ntff_profile_hook = get_axon_ntff_profile_hook
        sys.modules["antenv.axon_hooks"] = mod
        antenv.axon_hooks = mod
        from trn_agent_boot.trn_boot import _ntff_profile_via_ctypes
        mod._hook = _ntff_profile_via_ctypes("/opt/axon/libaxon_pjrt.so")
        return True
    except Exception as e:  # pragma: no cover - profiling is best-effort
        print(f"ntff hook install failed: {e}")
        return False


def kernel(**inputs):
    global LAST_RESULT
    from concourse.bass_utils import run_bass_kernel_spmd
    if TRACE:
        _install_ntff_hook()

    G, pblk = _host_prep(
        inputs["atom_description"], inputs["coords"],
        inputs["mean"], inputs["std"], inputs["weight"])

    nc = _get_program()
    in_maps = [
        {"cx": np.ascontiguousarray(G[i * BPC:(i + 1) * BPC]),
         "pr": np.ascontiguousarray(pblk[i * BPC:(i + 1) * BPC])}
        for i in range(NCORES)
    ]
    res = run_bass_kernel_spmd(nc, in_maps, list(range(NCORES)), trace=TRACE)
    LAST_RESULT = res
    e = np.concatenate([res.results[i]["out"] for i in range(NCORES)], axis=0)
    e = e.astype(np.float32).reshape(NB, MC, MR)
    out = np.repeat(e[..., None], NALT, axis=-1)
    return np.ascontiguousarray(out.astype(np.float32))
